# revision 59
# baseline (speedup 1.0000x reference)
"""Trainium2 Bass kernel for GaussianSelfAttention (sparse 4-corner attention).

Math restructure (per batch b, S=197 tokens, D=768, P=196 patches):
  score[s,i] = k[idx[i,s-1]] . q[s]   (s>=1; row s=0 of the output is exactly
  ones and is assembled on the host).
  out[s] = sum_i softmax_i(score)[i] * v[idx[i,s-1]]

Device-side formulation (all heavy GEMMs on device, fp16 operands):
  * Per batch only the nu<=~50 distinct gathered token rows matter. Batches
    are packed into NB=3 bins of <=128 gathered rows per core (bin batch
    counts fixed at compile time: 3,3,2). All t-dimensions live on the
    128-partition axis; the host ships x^T and xg^T pre-transposed so the
    PE does zero transposes.
  * QK[s,t'] = (X A Xg^T)[s,t'] with A = Wq Wk^T; q,k never materialize.
    s-only and constant bias terms cancel in the softmax; the t-dependent
    bias term and the 4-corner multiplicity counts are shipped as
    lnct = ln(count) + r2 (-30000 for zeros) and added into the QK PSUM
    accumulation through one identity-stationary matmul, so
    wu = exp(QK + lnct) comes straight off the Activation engine as f32r.
  * V = Xg @ Wv per bin; out = (wu^T @ [v|1]) as numerator / Z per 128-row
    output tile (reciprocal + per-partition scaled copies, batched per bin
    so the engine queues pipeline instead of chaining per tile).
  * xg^T and A^T ride in one merged DRAM tensor so each per-kd DMA is big
    enough to hide the fixed HWDGE descriptor-generation cost; adjacent
    full output tiles flush through one DMA for the same reason.
  * Output is written fp16 (196 patch rows per batch); the host adds bv,
    prepends the exact ones row for the class token and casts to f32.

Sharding: data-parallel over batch, 8 batches per core on 8 cores.
"""

import sys

sys.path.insert(0, "/opt/trn_rl_repo")

import numpy as np

B, S, D, P = 64, 197, 768, 196
GRID = np.float32(14.0)
N_CORES = 8
BPC = B // N_CORES       # batches per core
KD = D // 128            # 6 contraction tiles
BIN_SIZES = (3, 3, 2)    # preferred batches-per-bin layout
BIN_FALLBACKS = [(3, 3, 2), (2, 2, 2, 2), (1,) * 8]
SCOLS = BPC * P          # 1568 s-columns per core (class token excluded)

_CACHE = {}


def _pack_bins(nus, bin_sizes):
    """Partition the per-core batches into bins with fixed batch counts so
    every bin's total distinct-token count is <= 128, minimizing the LAST
    bin's fill (it bounds the M1 moving width). Returns (bins, w_last) or
    None if infeasible."""
    import itertools
    n = len(nus)
    idxs = list(range(n))
    best = None
    for gl in itertools.combinations(idxs, bin_sizes[-1]):
        sl = sum(nus[i] for i in gl)
        if sl > 128:
            continue
        rest = [i for i in idxs if i not in gl]

        def split(rem, sizes):
            if not sizes:
                return []
            for g in itertools.combinations(rem, sizes[0]):
                if sum(nus[i] for i in g) > 128:
                    continue
                sub = split([i for i in rem if i not in g], sizes[1:])
                if sub is not None:
                    return [list(g)] + sub
            return None
        lead = split(rest, list(bin_sizes[:-1]))
        if lead is None:
            continue
        if best is None or sl < best[1]:
            best = (lead + [list(gl)], sl)
    return best


def _host_precompute(x, norm_x, norm_y, Wq, bq, Wk, bk, Wv, bv, avgs, std_devs,
                     img_ids, mask):
    """Replicates the reference's index math exactly in float32 numpy and
    builds the fp16 device tensors."""
    f32, f16 = np.float32, np.float16
    x = np.asarray(x, f32)
    Wq = np.asarray(Wq, f32)
    Wk = np.asarray(Wk, f32)
    Wv = np.asarray(Wv, f32)
    bq = np.asarray(bq, f32)
    bv = np.asarray(bv, f32)

    mu = np.asarray(avgs, f32)[np.asarray(img_ids)]
    sd = np.asarray(std_devs, f32)[np.asarray(img_ids)]
    kx = (np.asarray(norm_x, f32) - mu[:, 0]) / sd[:, 0]
    ky = (np.asarray(norm_y, f32) - mu[:, 1]) / sd[:, 1]
    kx1, kx2 = np.ceil(kx), np.floor(kx)
    ky1, ky2 = np.ceil(ky), np.floor(ky)
    idx_f = np.stack([GRID * ky1 + kx1, GRID * ky1 + kx2,
                      GRID * ky2 + kx1, GRID * ky2 + kx2], axis=1)  # (B,4,P)
    idx = idx_f.astype(np.int32) % S  # trunc toward zero, then non-neg mod

    wb = Wk @ bq
    r2 = x @ wb                      # (B, S) t-dependent bias fold

    A = (Wq @ Wk.T).astype(f32)

    used_l, nu_l = [], []
    for b in range(B):
        u = np.unique(idx[b])
        used_l.append(u)
        nu_l.append(len(u))

    bin_sizes = None
    for cand in BIN_FALLBACKS:
        packs = []
        for c in range(N_CORES):
            nus = [nu_l[b] for b in range(c * BPC, (c + 1) * BPC)]
            packs.append(_pack_bins(nus, cand))
        if all(p is not None for p in packs):
            bin_sizes = cand
            break
    assert bin_sizes is not None, "cannot pack batches into any bin layout"

    TC = len(bin_sizes) * 128
    orders, bases, binid = [], [], []
    w_last = 0
    for c in range(N_CORES):
        bl = list(range(c * BPC, (c + 1) * BPC))
        nus = [nu_l[b] for b in bl]
        bins, wl = packs[c]
        w_last = max(w_last, wl)
        order, base, bid = [], [], []
        for k, g in enumerate(bins):
            off = 0
            for j in g:
                order.append(bl[j])
                base.append(off)
                bid.append(k)
                off += nus[j]
            assert off <= 128
        orders.append(order)
        bases.append(base)
        binid.append(bid)
    w_m1 = (len(bin_sizes) - 1) * 128 + ((w_last + 15) // 16 * 16)

    xt = np.zeros((N_CORES, 128, KD, SCOLS), f16)
    # merged xg^T | A^T tensor: per kd, [TC xg^T cols | 768 A^T cols]
    mg = np.zeros((N_CORES, 128, KD, TC + D), f16)
    lnct = np.full((N_CORES, 128, SCOLS), -30000.0, f16)
    at_h = np.ascontiguousarray(
        A.T.reshape(KD, 128, D).transpose(1, 0, 2)).astype(f16)
    wv_h = np.ascontiguousarray(
        Wv.reshape(KD, 128, D).transpose(1, 0, 2)).astype(f16)
    mg[:, :, :, TC:] = at_h[None]

    s_cols = np.tile(np.arange(P), 4)
    for c in range(N_CORES):
        for i, b in enumerate(orders[c]):
            k = binid[c][i]
            co = i * P
            u = used_l[b]
            nu = len(u)
            rb = bases[c][i]
            xt[c, :, :, co:co + P] = (
                x[b, 1:1 + P, :].T.reshape(KD, 128, P).transpose(1, 0, 2))
            mg[c, :, :, k * 128 + rb: k * 128 + rb + nu] = (
                x[b, u, :].T.reshape(KD, 128, nu).transpose(1, 0, 2))
            rank = np.zeros(S, np.int64)
            rank[u] = np.arange(nu)
            tp = rank[idx[b]]                       # (4, P)
            cb = np.zeros((128, P), f32)
            np.add.at(cb, (rb + tp.reshape(-1), s_cols), f32(1.0))
            with np.errstate(divide="ignore"):
                lcb = np.where(cb > 0, np.log(cb), f32(-30000.0))
            lcb[rb:rb + nu] += r2[b, u][:, None] * (cb[rb:rb + nu] > 0)
            lnct[c, :, co:co + P] = lcb

    meta = {"orders": orders, "w_m1": w_m1, "bin_sizes": bin_sizes}
    return xt, mg, lnct, wv_h, np.asarray(bv, f32).reshape(1, D), meta


# tuning knobs (best values found by sweeping TimelineSim)
TUNE = {
    "warm": 1,
    "stream": ["mg", "wva", "xt0", "wvb", "ln", "xt1", "xt2"],
    "order": ["v00", "v10", "v20", "qk0", "v01", "id0", "ex0", "ob0", "v11",
              "qk1", "id1", "ex1", "v21", "qk2", "id2", "ex2", "ob1", "ob2"],
    "vcopy": ["dve", "dve", "act", "dve", "act", "dve"],
    "m1copy": ["dve", "act", "dve", "act", "dve", "act"],
    "flips": (11,),           # tile indices whose o1/o2 engines swap
    "ypool": 6,
    "rpool": 8,
    "ydma": "sync",
    "ymerge": "12",           # bins whose full y tiles flush pairwise
    "chunkw": None,           # optional per-bin first QK chunk width
}


def _default_tune(nb):
    """Generalized emission/stream orders for a non-default bin count."""
    order = [f"v{k}0" for k in range(nb)]
    order += ["qk0", "v01", "id0", "ex0", "ob0"]
    for k in range(1, nb):
        order += [f"v{k}1", f"qk{k}", f"id{k}", f"ex{k}"]
    order += [f"ob{k}" for k in range(1, nb)]
    stream = ["mg", "wva", "xt0", "wvb", "ln"] + [
        f"xt{k}" for k in range(1, nb)]
    return {"order": order, "stream": stream, "flips": ()}


def _build_nc(bin_sizes=BIN_SIZES, w_m1=None, tune=None):
    import concourse.mybir as mybir
    import concourse.tile as tile
    from concourse import bacc
    from concourse.masks import make_identity
    from contextlib import ExitStack

    F32 = mybir.dt.float32
    F16 = mybir.dt.float16
    F32R = mybir.dt.float32r

    NB = len(bin_sizes)
    t = dict(TUNE)
    if NB != len(BIN_SIZES):
        t.update(_default_tune(NB))
    if tune:
        t.update(tune)

    TC = NB * 128
    if w_m1 is None:
        w_m1 = TC

    nc = bacc.Bacc("TRN2", target_bir_lowering=False, debug=False)

    xt_d = nc.dram_tensor("xt", [128, KD, SCOLS], F16, kind="ExternalInput")
    mg_d = nc.dram_tensor("mg", [128, KD, TC + D], F16, kind="ExternalInput")
    ln_d = nc.dram_tensor("lnct", [128, SCOLS], F16, kind="ExternalInput")
    wv_d = nc.dram_tensor("wv", [128, KD, D], F16, kind="ExternalInput")
    y_d = nc.dram_tensor("y", [SCOLS, D], F16, kind="ExternalOutput")

    bin_cols = [nb * P for nb in bin_sizes]
    bin_coff = [sum(bin_cols[:k]) for k in range(NB)]

    def chunks(k):
        w = bin_cols[k]
        if w * 4 <= 2048:
            return [(0, w)]
        h = t["chunkw"][k] if t.get("chunkw") else (w // 2 + 1) // 2 * 2
        out = []
        o = 0
        while o < w:
            cw = min(h if o == 0 else 512, w - o)
            out.append((o, cw))
            o += cw
        return out

    def stiles(k):
        out = []
        w = bin_cols[k]
        o = 0
        while o < w:
            sn = min(128, w - o)
            out.append((bin_coff[k] + o, sn))
            o += sn
        return out

    with tile.TileContext(nc) as tc:
        with ExitStack() as ctx:
            const = ctx.enter_context(tc.tile_pool(name="const", bufs=1))
            big = ctx.enter_context(tc.tile_pool(name="big", bufs=1))
            rpool = ctx.enter_context(tc.tile_pool(name="rpool", bufs=t["rpool"]))
            ypool = ctx.enter_context(tc.tile_pool(name="ypool", bufs=t["ypool"]))
            ps_a = ctx.enter_context(tc.tile_pool(name="ps_a", bufs=6, space="PSUM"))
            ps_b = ctx.enter_context(tc.tile_pool(name="ps_b", bufs=2, space="PSUM"))

            # ---- consts ----
            ident = const.tile([128, 128], F32)
            make_identity(nc, ident[:])
            ident16 = const.tile([128, 128], F16)
            nc.gpsimd.tensor_copy(ident16[:], ident[:])
            ones_f32 = const.tile([128, 2], F32)
            nc.vector.memset(ones_f32[:], 1.0)
            ones_r = const.tile([128, 2], F32R)
            nc.vector.tensor_copy(ones_r[:], ones_f32[:])
            dummy_mv = const.tile([128, 384], F16)
            nc.gpsimd.memset(dummy_mv[:], 0.0)

            # ---- input DMAs (single sync queue -> serial DMA engine) ----
            mg_sb = big.tile([128, KD, TC + D], F16)
            wv_sb = const.tile([128, KD, D], F16)
            xt_sb = big.tile([128, KD, SCOLS], F16)
            ln_sb = big.tile([128, SCOLS], F16)

            def emit_dma(tok):
                if tok == "mg":
                    for kd in range(KD):
                        nc.sync.dma_start(mg_sb[:, kd, :], mg_d[:, kd, :])
                elif tok == "wva":
                    nc.sync.dma_start(wv_sb[:, :, 0:384], wv_d[:, :, 0:384])
                elif tok == "wvb":
                    nc.sync.dma_start(wv_sb[:, :, 384:D], wv_d[:, :, 384:D])
                elif tok == "wv":
                    nc.sync.dma_start(wv_sb[:], wv_d[:, :, :])
                elif tok == "ln":
                    nc.sync.dma_start(ln_sb[:], ln_d[:, :])
                elif tok.startswith("xt"):
                    k = int(tok[2])
                    nc.sync.dma_start(
                        xt_sb[:, :, bin_coff[k]:bin_coff[k] + bin_cols[k]],
                        xt_d[:, :, bin_coff[k]:bin_coff[k] + bin_cols[k]])
                else:
                    raise ValueError(tok)

            for tok in t["stream"]:
                emit_dma(tok)

            m1_sb = big.tile([128, KD, TC], F16)
            if w_m1 < TC:
                nc.gpsimd.memset(m1_sb[:, :, w_m1:TC], 0.0)
            wu_sb = big.tile([128, SCOLS], F32R)
            v_sb = []
            for k in range(NB):
                vk = big.tile([128, D + 2], F32R, tag=f"v{k}", name=f"v{k}")
                v_sb.append(vk)

            # ---- PE warmup (p-state ramp) ----
            for w in range(t["warm"]):
                wp = ps_b.tile([128, 384], F32, tag="psb")
                nc.tensor.matmul(wp[:], ident16[:], dummy_mv[:],
                                 start=True, stop=True)

            # ---- M1 = A @ Xg^T, kd-outer across 6 psum banks ----
            m1ps = []
            for d2 in range(KD):
                mp = ps_a.tile([128, w_m1], F32, tag="psa", name=f"m1p{d2}")
                m1ps.append(mp)
            for kd in range(KD):
                for d2 in range(KD):
                    nc.tensor.matmul(
                        m1ps[d2][:],
                        mg_sb[:, kd, TC + 128 * d2:TC + 128 * (d2 + 1)],
                        mg_sb[:, kd, 0:w_m1],
                        start=(kd == 0), stop=(kd == KD - 1))
            for d2 in range(KD):
                if t["m1copy"][d2] == "dve":
                    nc.vector.tensor_copy(m1_sb[:, d2, 0:w_m1], m1ps[d2][:])
                else:
                    nc.scalar.copy(m1_sb[:, d2, 0:w_m1], m1ps[d2][:])

            vh_count = [0]

            def v_half(k, h):
                vp = ps_b.tile([128, 384], F32, tag="psb")
                for kd in range(KD):
                    nc.tensor.matmul(vp[:],
                                     mg_sb[:, kd, 128 * k:128 * (k + 1)],
                                     wv_sb[:, kd, 384 * h:384 * (h + 1)],
                                     start=(kd == 0), stop=(kd == KD - 1))
                eng = t["vcopy"][vh_count[0] % len(t["vcopy"])]
                vh_count[0] += 1
                if eng == "dve":
                    nc.vector.tensor_copy(v_sb[k][:, 384 * h:384 * (h + 1)],
                                          vp[:])
                else:
                    nc.scalar.copy(v_sb[k][:, 384 * h:384 * (h + 1)], vp[:])

            def qk_mms(k, c0, cw):
                qp = ps_a.tile([128, cw], F32, tag="psa", name=f"qk{k}_{c0}")
                for kd in range(KD):
                    nc.tensor.matmul(qp[:],
                                     m1_sb[:, kd, 128 * k:128 * (k + 1)],
                                     xt_sb[:, kd, bin_coff[k] + c0:
                                           bin_coff[k] + c0 + cw],
                                     start=(kd == 0), stop=False)
                return qp

            def qk_ident(k, c0, cw, qp):
                nc.tensor.matmul(qp[:], ident16[:],
                                 ln_sb[:, bin_coff[k] + c0:
                                       bin_coff[k] + c0 + cw],
                                 start=False, stop=True)

            def qk_exp(k, c0, cw, qp):
                nc.scalar.activation(
                    wu_sb[:, bin_coff[k] + c0: bin_coff[k] + c0 + cw],
                    qp[:], mybir.ActivationFunctionType.Exp)

            def out_mms(k, go, sn):
                o1 = ps_a.tile([sn, 512], F32, tag="psa", name=f"o1_{go}")
                nc.tensor.matmul(o1[:], wu_sb[:, go:go + sn],
                                 v_sb[k][:, 0:512], start=True, stop=True)
                o2 = ps_a.tile([sn, 258], F32, tag="psa", name=f"o2_{go}")
                nc.tensor.matmul(o2[:], wu_sb[:, go:go + sn],
                                 v_sb[k][:, 512:D + 2], start=True, stop=True)
                return o1, o2

            def out_recip(o2, sn):
                rz = rpool.tile([sn, 1], F32, tag="rz")
                nc.vector.reciprocal(rz[:], o2[:, 256:257])
                return rz

            def out_scale(yt_ap, o1, o2, rz, flip=False):
                if flip:
                    nc.vector.tensor_scalar_mul(yt_ap[:, 0:512], o1[:], rz[:])
                    nc.scalar.activation(yt_ap[:, 512:D], o2[:, 0:256],
                                         mybir.ActivationFunctionType.Copy,
                                         scale=rz[:])
                else:
                    nc.scalar.activation(yt_ap[:, 0:512], o1[:],
                                         mybir.ActivationFunctionType.Copy,
                                         scale=rz[:])
                    nc.vector.tensor_scalar_mul(yt_ap[:, 512:D], o2[:, 0:256],
                                                rz[:])

            def out_bin(k):
                tbase = sum(len(stiles(j)) for j in range(k))
                tiles = stiles(k)
                hands = []
                for (go, sn) in tiles:
                    hands.append(out_mms(k, go, sn))
                rzs = [out_recip(o2, sn)
                       for (o1, o2), (go, sn) in zip(hands, tiles)]
                # pair up adjacent full tiles so two tiles flush in one DMA
                merge_ok = (t["ymerge"] is True
                            or (isinstance(t["ymerge"], str)
                                and str(k) in t["ymerge"]))
                pairs = []
                i = 0
                while i < len(tiles):
                    if (merge_ok and i + 1 < len(tiles)
                            and tiles[i][1] == 128 and tiles[i + 1][1] == 128):
                        pairs.append((i, i + 1))
                        i += 2
                    else:
                        pairs.append((i,))
                        i += 1
                for pr in pairs:
                    if len(pr) == 2:
                        yt = ypool.tile([128, 2, D], F16, tag="yt2")
                        for j, i in enumerate(pr):
                            out_scale(yt[:, j, :], hands[i][0], hands[i][1],
                                      rzs[i], (tbase + i) in t["flips"])
                        go = tiles[pr[0]][0]
                        nc.sync.dma_start(
                            y_d[go:go + 256, :].rearrange(
                                "(j i) d -> i j d", j=2),
                            yt[:])
                    else:
                        i = pr[0]
                        go, sn = tiles[i]
                        yt = ypool.tile([sn, D], F16, tag="yt")
                        out_scale(yt[:], hands[i][0], hands[i][1], rzs[i],
                                  (tbase + i) in t["flips"])
                        nc.sync.dma_start(y_d[go:go + sn, :], yt[:])

            # ---- emission order (tuned against the timeline simulator) ----
            qps = {}

            def emit(tok):
                if tok.startswith("v"):
                    k, h = int(tok[1]), int(tok[2])
                    v_half(k, h)
                    if h == 1:
                        nc.gpsimd.tensor_copy(v_sb[k][:, D:D + 2], ones_r[:])
                elif tok.startswith("qk"):
                    k = int(tok[2])
                    qps[k] = [qk_mms(k, c0, cw) for (c0, cw) in chunks(k)]
                elif tok.startswith("id"):
                    k = int(tok[2])
                    for (c0, cw), qp in zip(chunks(k), qps[k]):
                        qk_ident(k, c0, cw, qp)
                elif tok.startswith("ex"):
                    k = int(tok[2])
                    for (c0, cw), qp in zip(chunks(k), qps[k]):
                        qk_exp(k, c0, cw, qp)
                elif tok.startswith("ob"):
                    out_bin(int(tok[2]))
                else:
                    raise ValueError(tok)

            for tok in t["order"]:
                emit(tok)

    nc.compile()
    return nc


def _get_nc(w_m1=None, bin_sizes=BIN_SIZES):
    if w_m1 is None:
        return _CACHE["last_nc"]
    key = ("nc", bin_sizes, w_m1)
    if key not in _CACHE:
        _CACHE[key] = _build_nc(bin_sizes, w_m1)
    _CACHE["last_nc"] = _CACHE[key]
    return _CACHE[key]


def kernel(x, norm_x, norm_y, Wq, bq, Wk, bk, Wv, bv, avgs, std_devs, img_ids,
           mask, _want_trace=False):
    from concourse.bass_utils import run_bass_kernel_spmd

    xt, mg, lnct, wv_h, bvr, meta = _host_precompute(
        x, norm_x, norm_y, Wq, bq, Wk, bk, Wv, bv, avgs, std_devs, img_ids, mask)

    in_maps = []
    for c in range(N_CORES):
        in_maps.append({
            "xt": np.ascontiguousarray(xt[c]),
            "mg": np.ascontiguousarray(mg[c]),
            "lnct": np.ascontiguousarray(lnct[c]),
            "wv": wv_h,
        })

    nc = _get_nc(meta["w_m1"], meta["bin_sizes"])
    res = run_bass_kernel_spmd(nc, in_maps, core_ids=list(range(N_CORES)),
                               trace=_want_trace)

    out = np.ones((B, S, D), np.float32)
    for c in range(N_CORES):
        yc = np.asarray(res.results[c]["y"], np.float32)   # (SCOLS, D)
        for i, b in enumerate(meta["orders"][c]):
            out[b, 1:1 + P, :] = yc[i * P:(i + 1) * P, :]
    if np.any(bvr):
        out[:, 1:, :] += bvr[0]
    if _want_trace:
        _CACHE["last_result"] = res
    return out
